# revision 8
# baseline (speedup 1.0000x reference)
"""AttLIF Trainium2 kernel: Linear(1024->2048) + temporal-attention gate + IF scan.

Self-contained: B=256, T=64, DIN=1024, DH=2048, 8 NeuronCores, data-parallel
over batch (BS=32 per core). Per core, groups of bg=8 batches:

  GEMM    x[bt,h] = dE[bt,k] @ WE[k,h]   bf16 3-term hi/lo split (K=3073->3200)
          stationary = data k-tiles, moving = weight 512-chunks, PSUM f32
  avg     ACT engine computes it for free: PSUM->SBUF copy with accum_out
  mx      DVE reduce over 512-chunks, then over chunks
  score   sigmoid(W2 @ (relu(W1@avg) + relu(W1@mx)))  tiny PE matmuls
  scan    u = x*score + v; v = u*(u<VTH)  2 DVE stt ops per t
  spike   u >= VTH -> uint8, dumped in raw scan layout; host decodes

Layouts are host-prepped so every HBM load is contiguous per partition.
The (b,t)xh -> (h,b)x(t,h') rearrange runs as 64KB DMAs round-robined over
the sync/scalar/gpsimd queues, overlapped with the GEMM of the next chunk.
"""
import os
import sys
from contextlib import ExitStack

import numpy as np

sys.path.insert(0, "/opt/trn_rl_repo")

VTH = 0.6
B, T, DIN, DH = 256, 64, 1024, 2048
NCORES = 8
BS = B // NCORES     # 32
NM = BS * T // 128   # 16 m-tiles per core
KE = 3 * DIN + 1     # bf16x3 extended contraction (+1 bias row)
NK = (KE + 127) // 128   # 25
KPAD = NK * 128      # 3200
NN = DH // 512       # 4 n-chunks
BG = 8               # batches per scan group
MG = BG // 2         # 4 m-tiles per group
NG = BS // BG        # 4 groups
JW = BG * DH // 128  # 128 free elems per t in scan layout
HH = DH // JW        # 16 h_hi values


def _prep_weights(W, bias, W1, W2):
    import ml_dtypes
    bf = ml_dtypes.bfloat16
    Whi32 = W.astype(bf).astype(np.float32)
    Wlo = (W - Whi32).astype(bf).astype(np.float32)
    WE = np.concatenate([Whi32.T, Wlo.T, Whi32.T, bias[None, :]], axis=0)
    WEp = np.zeros((KPAD, DH), np.float32)
    WEp[:KE] = WE
    # per n-chunk: [kp, k*512+j] contiguous per partition
    wT = np.ascontiguousarray(
        WEp.reshape(NK, 128, NN, 512).transpose(2, 1, 0, 3)
        .reshape(NN, 128, NK * 512)).astype(bf)
    w1t = np.ascontiguousarray(W1.T).astype(np.float32)
    w2t = np.ascontiguousarray(W2.T).astype(np.float32)
    return dict(wT=wT, w1t=w1t, w2t=w2t)


def _prep_data_shard(shard):
    import ml_dtypes
    bf = ml_dtypes.bfloat16
    rows = shard.reshape(BS * T, DIN).astype(np.float32)
    dhi32 = rows.astype(bf).astype(np.float32)
    dlo = (rows - dhi32).astype(bf).astype(np.float32)
    dE = np.concatenate(
        [dhi32, dhi32, dlo, np.ones((BS * T, 1), np.float32)], axis=1)
    dEp = np.zeros((BS * T, KPAD), np.float32)
    dEp[:, :dE.shape[1]] = dE
    # per m-tile: [kp, k*128+m] contiguous per partition
    return np.ascontiguousarray(
        dEp.reshape(NM, 128, NK, 128).transpose(0, 3, 2, 1)
        .reshape(NM, 128, NK * 128)).astype(bf)


def _decode_out(dump):
    # dump u8 [NG, 128, T*JW]; partition p = hh*BG + bl, free = t*JW + j
    a = dump.reshape(NG, HH, BG, T, JW).transpose(0, 2, 3, 1, 4)
    return np.ascontiguousarray(a).reshape(BS, T, DH).astype(np.float32)


def _build(nc, tile, mybir):
    f32 = mybir.dt.float32
    bf16 = mybir.dt.bfloat16
    u8 = mybir.dt.uint8
    aop = mybir.AluOpType

    dT = nc.dram_tensor("dT", [NM, 128, NK * 128], bf16, kind="ExternalInput").ap()
    wT = nc.dram_tensor("wT", [NN, 128, NK * 512], bf16, kind="ExternalInput").ap()
    w1t = nc.dram_tensor("w1t", [T, 4], f32, kind="ExternalInput").ap()
    w2t = nc.dram_tensor("w2t", [4, T], f32, kind="ExternalInput").ap()
    outD = nc.dram_tensor("out", [NG, 128, T * JW], u8, kind="ExternalOutput").ap()

    with tile.TileContext(nc) as tc, ExitStack() as ctx:
        cpool = ctx.enter_context(tc.tile_pool(name="cpool", bufs=1))
        wpool = ctx.enter_context(tc.tile_pool(name="wpool", bufs=2))
        dpool = ctx.enter_context(tc.tile_pool(name="dpool", bufs=MG + 4))
        xmpool = ctx.enter_context(tc.tile_pool(name="xmpool", bufs=4))
        xspool = ctx.enter_context(tc.tile_pool(name="xspool", bufs=2))
        stpool = ctx.enter_context(tc.tile_pool(name="stpool", bufs=2))
        scpool = ctx.enter_context(tc.tile_pool(name="scpool", bufs=2))
        vpool = ctx.enter_context(tc.tile_pool(name="vpool", bufs=2))
        opool = ctx.enter_context(tc.tile_pool(name="opool", bufs=2))
        pgemm = ctx.enter_context(tc.tile_pool(name="pgemm", bufs=4, space="PSUM"))
        pmisc = ctx.enter_context(tc.tile_pool(name="pmisc", bufs=1, space="PSUM"))

        w1t_sb = cpool.tile([128, 4], f32, name="w1t_sb")
        nc.sync.dma_start(w1t_sb[0:T, :], w1t[:])
        nc.sync.dma_start(w1t_sb[T:128, :], w1t[:])
        w2t_sb = cpool.tile([4, T], f32, name="w2t_sb")
        nc.sync.dma_start(w2t_sb[:], w2t[:])



        # Software-pipelined over groups: while group g's GEMM chunks are
        # emitted, group g-1's scan steps are interleaved on the DVE queue
        # (4 t-steps per chunk slot) so g's rmx reduces never queue behind a
        # full 40us scan and xm/PSUM pools keep draining.
        prev = None   # ((XS, ssc, v), g_idx) whose scan is pending

        def emit_scan_steps(state, t0, t1):
            XS_p, ssc_p, v_p = state
            for t in range(t0, t1):
                xt = XS_p[:, t * JW:(t + 1) * JW]
                nc.vector.scalar_tensor_tensor(
                    xt, xt, ssc_p[:, t:t + 1], v_p[:], op0=aop.mult, op1=aop.add)
                nc.vector.scalar_tensor_tensor(
                    v_p[:], xt, VTH, xt, op0=aop.is_lt, op1=aop.mult)

        def emit_spike_store(state, g_idx):
            XS_p, _, _ = state
            osb = opool.tile([128, T * JW], u8, name="osb", tag="osb")
            half = T * JW // 2
            for piece in range(2):
                nc.vector.tensor_scalar(
                    osb[:, piece * half:(piece + 1) * half],
                    XS_p[:, piece * half:(piece + 1) * half],
                    VTH, None, op0=aop.is_ge)
            nc.sync.dma_start(outD[g_idx], osb[:])

        NSLOT = NN * MG
        TPS = T // NSLOT   # t-steps of the previous scan drained per slot

        for g in range(NG + 1):
            last = g == NG
            if not last:
                XS = xspool.tile([128, T * JW], f32, name="XS", tag="XS")
                asum = stpool.tile([128, MG * NN], f32, name="asum", tag="asum")
                rmx = stpool.tile([128, MG * NN], f32, name="rmx", tag="rmx")
                stats = stpool.tile([128, 2 * MG], f32, name="stats", tag="stats")
                dts = []
                for ml in range(MG):
                    dt = dpool.tile([128, NK * 128], bf16, name="dt", tag="dt")
                    nc.sync.dma_start(dt[:], dT[g * MG + ml])
                    dts.append(dt)
            slot = 0
            for n in range(NN):
                if not last:
                    wc = wpool.tile([128, NK * 512], bf16, name="wc", tag="wc")
                    nc.gpsimd.dma_start(wc[:], wT[n])
                for ml in range(MG):
                    if not last:
                        dt = dts[ml]
                        ps = pgemm.tile([128, 512], f32, name="ps", tag="ps")
                        for k in range(NK):
                            nc.tensor.matmul(ps[:], dt[:, k * 128:(k + 1) * 128],
                                             wc[:, k * 512:(k + 1) * 512],
                                             start=(k == 0), stop=(k == NK - 1))
                        xm = xmpool.tile([128, 512], f32, name="xm", tag="xm")
                        c = ml * NN + n
                        # PSUM->SBUF copy; ACT also emits the h-chunk sum
                        nc.scalar.activation(
                            xm[:], ps[:], mybir.ActivationFunctionType.Copy,
                            accum_out=asum[:, c:c + 1])
                        nc.vector.tensor_reduce(
                            rmx[:, c:c + 1], xm[:], mybir.AxisListType.X, aop.max)
                        # scan layout: p = h_hi*BG + b_l, free = t*JW + j
                        # SWDGE trigger ~1us; transfer runs async on SDMA rings
                        for h2 in range(4):
                            p0 = (n * 4 + h2) * BG + 2 * ml
                            nc.gpsimd.dma_start(XS[p0:p0 + 2, :],
                                                xm[:, h2 * JW:(h2 + 1) * JW])
                    if prev is not None:
                        emit_scan_steps(prev[0], slot * TPS, (slot + 1) * TPS)
                    slot += 1
            if prev is not None:
                emit_spike_store(prev[0], prev[1])
                prev = None
            if last:
                break
            # stats: avg = sum(asum)/DH, mx = max(rmx) per m-tile column
            nc.vector.tensor_reduce(
                stats[:, 0:MG], asum[:].rearrange("p (m n) -> p m n", n=NN),
                mybir.AxisListType.X, aop.add)
            nc.vector.tensor_scalar(
                stats[:, 0:MG], stats[:, 0:MG], 1.0 / DH, None, op0=aop.mult)
            nc.vector.tensor_reduce(
                stats[:, MG:2 * MG], rmx[:].rearrange("p (m n) -> p m n", n=NN),
                mybir.AxisListType.X, aop.max)

            h1a = pmisc.tile([4, 2 * MG], f32, name="h1a", tag="pm1")
            nc.tensor.matmul(h1a[:], w1t_sb[0:T, :], stats[0:T, :],
                             start=True, stop=True)
            h1b = pmisc.tile([4, 2 * MG], f32, name="h1b", tag="pm2")
            nc.tensor.matmul(h1b[:], w1t_sb[T:128, :], stats[T:128, :],
                             start=True, stop=True)
            h1r = scpool.tile([4, 4 * MG], f32, name="h1r", tag="h1r")
            nc.scalar.activation(h1r[:, 0:2 * MG], h1a[:],
                                 mybir.ActivationFunctionType.Relu)
            nc.scalar.activation(h1r[:, 2 * MG:4 * MG], h1b[:],
                                 mybir.ActivationFunctionType.Relu)
            # Ht columns in natural batch order b_l = 2*ml + b2
            Ht = scpool.tile([4, 2 * MG], f32, name="Ht", tag="Ht")
            h4 = h1r[:].rearrange("r (b s m) -> r b s m", b=2, s=2)
            nc.vector.tensor_tensor(
                Ht[:].rearrange("r (m b) -> r b m", b=2), h4[:, :, 0], h4[:, :, 1],
                aop.add)
            spT = pmisc.tile([2 * MG, T], f32, name="spT", tag="pm1")
            nc.tensor.matmul(spT[:], Ht[:], w2t_sb[:], start=True, stop=True)
            scb = scpool.tile([2 * MG, T], f32, name="scb", tag="scb")
            nc.scalar.activation(scb[:], spT[:],
                                 mybir.ActivationFunctionType.Sigmoid)
            ssc = scpool.tile([128, T], f32, name="ssc", tag="ssc")
            nc.scalar.dma_start(ssc[0:BG, :], scb[:])
            for m in (1, 2, 4, 8):   # log-doubling partition replicate
                nc.scalar.dma_start(ssc[m * BG:2 * m * BG, :], ssc[0:m * BG, :])

            v = vpool.tile([128, JW], f32, name="v", tag="v")
            nc.vector.memset(v[:], 0.0)
            prev = ((XS, ssc, v), g)


_CACHE = {}


def _get_compiled():
    if "nc" in _CACHE:
        return _CACHE["nc"]
    import concourse.tile as tile
    from concourse import bacc, mybir
    nc = bacc.Bacc("TRN2", target_bir_lowering=False, debug=False, num_devices=1)
    _build(nc, tile, mybir)
    nc.compile()
    _CACHE["nc"] = nc
    return nc


def kernel(data, W, bias, W1, W2):
    from concourse.bass_utils import run_bass_kernel_spmd

    data = np.asarray(data, dtype=np.float32)
    W = np.asarray(W, dtype=np.float32)
    bias = np.asarray(bias, dtype=np.float32)
    W1 = np.asarray(W1, dtype=np.float32)
    W2 = np.asarray(W2, dtype=np.float32)

    wargs = _prep_weights(W, bias, W1, W2)
    in_maps = []
    for c in range(NCORES):
        shard = data[c * BS:(c + 1) * BS]
        in_maps.append({"dT": _prep_data_shard(shard), **wargs})

    nc = _get_compiled()
    res = run_bass_kernel_spmd(nc, in_maps, core_ids=list(range(NCORES)))
    outs = [_decode_out(res.results[c]["out"]) for c in range(NCORES)]
    return np.concatenate(outs, axis=0)


if __name__ == "__main__":
    rng = np.random.default_rng(0)
    d = rng.standard_normal((B, T, DIN)).astype(np.float32)
    w = (rng.standard_normal((DH, DIN)) / 32.0).astype(np.float32)
    b = np.zeros(DH, np.float32)
    w1 = (rng.standard_normal((4, T)) / 8.0).astype(np.float32)
    w2 = (rng.standard_normal((T, 4)) / 2.0).astype(np.float32)
    o = kernel(d, w, b, w1, w2)
    print(o.shape, o.dtype, o.mean())


# revision 9
# speedup vs baseline: 1.2398x; 1.2398x over previous
"""AttLIF Trainium2 kernel: Linear(1024->2048) + temporal-attention gate + IF scan.

Self-contained: B=256, T=64, DIN=1024, DH=2048, 8 NeuronCores, data-parallel
over batch (BS=32 per core). Per core, groups of bg=8 batches:

  GEMM    x[bt,h] = dE[bt,k] @ WE[k,h]   bf16 3-term hi/lo split (K=3073->3200)
          stationary = data k-tiles, moving = weight 512-chunks, PSUM f32
  avg     ACT engine computes it for free: PSUM->SBUF copy with accum_out
  mx      DVE reduce over 512-chunks, then over chunks
  score   sigmoid(W2 @ (relu(W1@avg) + relu(W1@mx)))  tiny PE matmuls
  scan    u = x*score + v; v = u*(u<VTH)  2 DVE stt ops per t
  spike   u >= VTH -> uint8, dumped in raw scan layout; host decodes

Layouts are host-prepped so every HBM load is contiguous per partition.
The (b,t)xh -> (h,b)x(t,h') rearrange runs as 64KB DMAs round-robined over
the sync/scalar/gpsimd queues, overlapped with the GEMM of the next chunk.
"""
import os
import sys
from contextlib import ExitStack

import numpy as np

sys.path.insert(0, "/opt/trn_rl_repo")

VTH = 0.6
B, T, DIN, DH = 256, 64, 1024, 2048
NCORES = 8
BS = B // NCORES     # 32
NM = BS * T // 128   # 16 m-tiles per core
KE = 3 * DIN + 1     # bf16x3 extended contraction (+1 bias row)
NK = (KE + 127) // 128   # 25
KPAD = NK * 128      # 3200
NN = DH // 512       # 4 n-chunks
BG = 8               # batches per scan group
MG = BG // 2         # 4 m-tiles per group
NG = BS // BG        # 4 groups
JW = BG * DH // 128  # 128 free elems per t in scan layout
HH = DH // JW        # 16 h_hi values


def _prep_weights(W, bias, W1, W2):
    import ml_dtypes
    bf = ml_dtypes.bfloat16
    Whi32 = W.astype(bf).astype(np.float32)
    Wlo = (W - Whi32).astype(bf).astype(np.float32)
    WE = np.concatenate([Whi32.T, Wlo.T, Whi32.T, bias[None, :]], axis=0)
    WEp = np.zeros((KPAD, DH), np.float32)
    WEp[:KE] = WE
    # per n-chunk: [kp, k*512+j] contiguous per partition
    wT = np.ascontiguousarray(
        WEp.reshape(NK, 128, NN, 512).transpose(2, 1, 0, 3)
        .reshape(NN, 128, NK * 512)).astype(bf)
    w1t = np.ascontiguousarray(W1.T).astype(np.float32)
    w2t = np.ascontiguousarray(W2.T).astype(np.float32)
    return dict(wT=wT, w1t=w1t, w2t=w2t)


def _prep_data_shard(shard):
    import ml_dtypes
    bf = ml_dtypes.bfloat16
    rows = shard.reshape(BS * T, DIN).astype(np.float32)
    dhi32 = rows.astype(bf).astype(np.float32)
    dlo = (rows - dhi32).astype(bf).astype(np.float32)
    dE = np.concatenate(
        [dhi32, dhi32, dlo, np.ones((BS * T, 1), np.float32)], axis=1)
    dEp = np.zeros((BS * T, KPAD), np.float32)
    dEp[:, :dE.shape[1]] = dE
    # per m-tile: [kp, k*128+m] contiguous per partition
    return np.ascontiguousarray(
        dEp.reshape(NM, 128, NK, 128).transpose(0, 3, 2, 1)
        .reshape(NM, 128, NK * 128)).astype(bf)


def _decode_out(dump):
    # dump u8 [NG, 128, T*JW]; partition p = hh*BG + bl, free = t*JW + j
    a = dump.reshape(NG, HH, BG, T, JW).transpose(0, 2, 3, 1, 4)
    return np.ascontiguousarray(a).reshape(BS, T, DH).astype(np.float32)


def _build(nc, tile, mybir):
    f32 = mybir.dt.float32
    bf16 = mybir.dt.bfloat16
    u8 = mybir.dt.uint8
    aop = mybir.AluOpType

    dT = nc.dram_tensor("dT", [NM, 128, NK * 128], bf16, kind="ExternalInput").ap()
    wT = nc.dram_tensor("wT", [NN, 128, NK * 512], bf16, kind="ExternalInput").ap()
    w1t = nc.dram_tensor("w1t", [T, 4], f32, kind="ExternalInput").ap()
    w2t = nc.dram_tensor("w2t", [4, T], f32, kind="ExternalInput").ap()
    outD = nc.dram_tensor("out", [NG, 128, T * JW], u8, kind="ExternalOutput").ap()

    with tile.TileContext(nc) as tc, ExitStack() as ctx:
        cpool = ctx.enter_context(tc.tile_pool(name="cpool", bufs=1))
        wpool = ctx.enter_context(tc.tile_pool(name="wpool", bufs=2))
        dpool = ctx.enter_context(tc.tile_pool(name="dpool", bufs=MG + 4))
        xmpool = ctx.enter_context(tc.tile_pool(name="xmpool", bufs=4))
        xspool = ctx.enter_context(tc.tile_pool(name="xspool", bufs=2))
        stpool = ctx.enter_context(tc.tile_pool(name="stpool", bufs=2))
        scpool = ctx.enter_context(tc.tile_pool(name="scpool", bufs=2))
        vpool = ctx.enter_context(tc.tile_pool(name="vpool", bufs=2))
        opool = ctx.enter_context(tc.tile_pool(name="opool", bufs=2))
        pgemm = ctx.enter_context(tc.tile_pool(name="pgemm", bufs=4, space="PSUM"))
        pmisc = ctx.enter_context(tc.tile_pool(name="pmisc", bufs=1, space="PSUM"))

        w1t_sb = cpool.tile([128, 4], f32, name="w1t_sb")
        nc.sync.dma_start(w1t_sb[0:T, :], w1t[:])
        nc.sync.dma_start(w1t_sb[T:128, :], w1t[:])
        w2t_sb = cpool.tile([4, T], f32, name="w2t_sb")
        nc.sync.dma_start(w2t_sb[:], w2t[:])



        # Software-pipelined over groups. Group g's chunk slots also emit,
        # one per slot, deferred work of group g-1: stats -> score MLP ->
        # sigmoid/ssc -> the 64-step scan (5 t-steps/slot). This keeps the
        # score matmuls off the PE queue head at group boundaries and spreads
        # the DVE scan between the rmx reduces. Weight/data loads for the
        # next n-chunk/group are emitted one step early so their DMAs run
        # under the current GEMM.
        def mk_chunk_sched(XS_, asum_, rmx_, stats_, g_):
            """Deferred-emission closures for group g_, one per slot of g_+1."""
            st = {}

            def s_stats():
                nc.vector.tensor_reduce(
                    stats_[:, 0:MG],
                    asum_[:].rearrange("p (m n) -> p m n", n=NN),
                    mybir.AxisListType.X, aop.add)
                nc.vector.tensor_scalar(
                    stats_[:, 0:MG], stats_[:, 0:MG], 1.0 / DH, None,
                    op0=aop.mult)
                nc.vector.tensor_reduce(
                    stats_[:, MG:2 * MG],
                    rmx_[:].rearrange("p (m n) -> p m n", n=NN),
                    mybir.AxisListType.X, aop.max)

            def s_mlp1():
                h1a = pmisc.tile([4, 2 * MG], f32, name="h1a", tag="pm1")
                nc.tensor.matmul(h1a[:], w1t_sb[0:T, :], stats_[0:T, :],
                                 start=True, stop=True)
                h1b = pmisc.tile([4, 2 * MG], f32, name="h1b", tag="pm2")
                nc.tensor.matmul(h1b[:], w1t_sb[T:128, :], stats_[T:128, :],
                                 start=True, stop=True)
                h1r = scpool.tile([4, 4 * MG], f32, name="h1r", tag="h1r")
                nc.scalar.activation(h1r[:, 0:2 * MG], h1a[:],
                                     mybir.ActivationFunctionType.Relu)
                nc.scalar.activation(h1r[:, 2 * MG:4 * MG], h1b[:],
                                     mybir.ActivationFunctionType.Relu)
                st["h1r"] = h1r

            def s_mlp2():
                h1r = st["h1r"]
                # Ht columns in natural batch order b_l = 2*ml + b2
                Ht = scpool.tile([4, 2 * MG], f32, name="Ht", tag="Ht")
                h4 = h1r[:].rearrange("r (b s m) -> r b s m", b=2, s=2)
                nc.vector.tensor_tensor(
                    Ht[:].rearrange("r (m b) -> r b m", b=2),
                    h4[:, :, 0], h4[:, :, 1], aop.add)
                spT = pmisc.tile([2 * MG, T], f32, name="spT", tag="pm1")
                nc.tensor.matmul(spT[:], Ht[:], w2t_sb[:], start=True, stop=True)
                scb = scpool.tile([2 * MG, T], f32, name="scb", tag="scb")
                nc.scalar.activation(scb[:], spT[:],
                                     mybir.ActivationFunctionType.Sigmoid)
                ssc = scpool.tile([128, T], f32, name="ssc", tag="ssc")
                nc.scalar.dma_start(ssc[0:BG, :], scb[:])
                for m in (1, 2, 4, 8):   # log-doubling partition replicate
                    nc.scalar.dma_start(ssc[m * BG:2 * m * BG, :],
                                        ssc[0:m * BG, :])
                v = vpool.tile([128, JW], f32, name="v", tag="v")
                nc.vector.memset(v[:], 0.0)
                st["ssc"], st["v"] = ssc, v

            def mk_scan(t0, t1):
                def s_scan():
                    ssc, v = st["ssc"], st["v"]
                    for t in range(t0, t1):
                        xt = XS_[:, t * JW:(t + 1) * JW]
                        nc.vector.scalar_tensor_tensor(
                            xt, xt, ssc[:, t:t + 1], v[:],
                            op0=aop.mult, op1=aop.add)
                        nc.vector.scalar_tensor_tensor(
                            v[:], xt, VTH, xt, op0=aop.is_lt, op1=aop.mult)
                return s_scan

            sched = [s_stats, s_mlp1, s_mlp2]
            nsl = NSLOT - 3
            done = 0
            for i in range(nsl):
                t1 = min(T, done + (T + nsl - 1) // nsl)
                sched.append(mk_scan(done, t1))
                done = t1

            def s_spike():
                osb = opool.tile([128, T * JW], u8, name="osb", tag="osb")
                half = T * JW // 2
                for piece in range(2):
                    nc.vector.tensor_scalar(
                        osb[:, piece * half:(piece + 1) * half],
                        XS_[:, piece * half:(piece + 1) * half],
                        VTH, None, op0=aop.is_ge)
                nc.sync.dma_start(outD[g_], osb[:])

            return sched, s_spike

        NSLOT = NN * MG
        sched, spike_fn = [], None
        wc_cur = None

        def load_wc(n):
            w = wpool.tile([128, NK * 512], bf16, name="wc", tag="wc")
            nc.sync.dma_start(w[:], wT[n])
            return w

        def load_dts(g):
            out = []
            for ml in range(MG):
                dt = dpool.tile([128, NK * 128], bf16, name="dt", tag="dt")
                nc.sync.dma_start(dt[:], dT[g * MG + ml])
                out.append(dt)
            return out

        dts = load_dts(0)
        wc_next = load_wc(0)
        dts_next = None

        for g in range(NG + 1):
            last = g == NG
            if not last:
                XS = xspool.tile([128, T * JW], f32, name="XS", tag="XS")
                asum = stpool.tile([128, MG * NN], f32, name="asum", tag="asum")
                rmx = stpool.tile([128, MG * NN], f32, name="rmx", tag="rmx")
                stats = stpool.tile([128, 2 * MG], f32, name="stats", tag="stats")
            slot = 0
            for n in range(NN):
                if not last:
                    wc_cur = wc_next
                    # prefetch next n-chunk (cyclic into the next group)
                    if n < NN - 1:
                        wc_next = load_wc(n + 1)
                    elif g < NG - 1:
                        wc_next = load_wc(0)
                    if n == 2 and g < NG - 1:
                        dts_next = load_dts(g + 1)
                for ml in range(MG):
                    if not last:
                        dt = dts[ml]
                        ps = pgemm.tile([128, 512], f32, name="ps", tag="ps")
                        for k in range(NK):
                            nc.tensor.matmul(ps[:], dt[:, k * 128:(k + 1) * 128],
                                             wc_cur[:, k * 512:(k + 1) * 512],
                                             start=(k == 0), stop=(k == NK - 1))
                        xm = xmpool.tile([128, 512], f32, name="xm", tag="xm")
                        c = ml * NN + n
                        # PSUM->SBUF copy; ACT also emits the h-chunk sum
                        nc.scalar.activation(
                            xm[:], ps[:], mybir.ActivationFunctionType.Copy,
                            accum_out=asum[:, c:c + 1])
                        nc.vector.tensor_reduce(
                            rmx[:, c:c + 1], xm[:], mybir.AxisListType.X, aop.max)
                        # scan layout: p = h_hi*BG + b_l, free = t*JW + j
                        # SWDGE trigger ~1us; transfer runs async on SDMA rings
                        for h2 in range(4):
                            p0 = (n * 4 + h2) * BG + 2 * ml
                            nc.gpsimd.dma_start(XS[p0:p0 + 2, :],
                                                xm[:, h2 * JW:(h2 + 1) * JW])
                    if slot < len(sched):
                        sched[slot]()
                    slot += 1
            for i in range(slot, len(sched)):   # drain (last iteration)
                sched[i]()
            if spike_fn is not None:
                spike_fn()
            if last:
                break
            sched, spike_fn = mk_chunk_sched(XS, asum, rmx, stats, g)
            if dts_next is not None:
                dts, dts_next = dts_next, None


_CACHE = {}


def _get_compiled():
    if "nc" in _CACHE:
        return _CACHE["nc"]
    import concourse.tile as tile
    from concourse import bacc, mybir
    nc = bacc.Bacc("TRN2", target_bir_lowering=False, debug=False, num_devices=1)
    _build(nc, tile, mybir)
    nc.compile()
    _CACHE["nc"] = nc
    return nc


def kernel(data, W, bias, W1, W2):
    from concourse.bass_utils import run_bass_kernel_spmd

    data = np.asarray(data, dtype=np.float32)
    W = np.asarray(W, dtype=np.float32)
    bias = np.asarray(bias, dtype=np.float32)
    W1 = np.asarray(W1, dtype=np.float32)
    W2 = np.asarray(W2, dtype=np.float32)

    wargs = _prep_weights(W, bias, W1, W2)
    in_maps = []
    for c in range(NCORES):
        shard = data[c * BS:(c + 1) * BS]
        in_maps.append({"dT": _prep_data_shard(shard), **wargs})

    nc = _get_compiled()
    res = run_bass_kernel_spmd(nc, in_maps, core_ids=list(range(NCORES)))
    outs = [_decode_out(res.results[c]["out"]) for c in range(NCORES)]
    return np.concatenate(outs, axis=0)


if __name__ == "__main__":
    rng = np.random.default_rng(0)
    d = rng.standard_normal((B, T, DIN)).astype(np.float32)
    w = (rng.standard_normal((DH, DIN)) / 32.0).astype(np.float32)
    b = np.zeros(DH, np.float32)
    w1 = (rng.standard_normal((4, T)) / 8.0).astype(np.float32)
    w2 = (rng.standard_normal((T, 4)) / 2.0).astype(np.float32)
    o = kernel(d, w, b, w1, w2)
    print(o.shape, o.dtype, o.mean())
